# revision 25
# baseline (speedup 1.0000x reference)
"""KMeans assignment kernel for TRN2 (8 NeuronCores, data-parallel over points).

Computes argmin_k ||x_n - c_k||^2 for x (65536, 512) f32, centers (4096, 512) f32.

v5 strategy (single-pass fp32r + PSUM-prefill bias + tournament argmax):
  - argmin_k dist = argmax_k s,  s = 2x.c_k - ||c_k||^2  (x-norm constant/row).
  - ONE fp32r matmul pass (e8m11 operands): ~13/65536 argmin flips
    (rel err ~9e-3, gate 2e-2) at 3x less PE work than exact 3-pass.
  - -||c||^2 is PREFILLED into PSUM by ScalarE; matmuls accumulate onto it
    (start=False; has_written bits primed once), so s materializes in PSUM.
  - Egress PSUM->SBUF split between ScalarE and VectorE for balance.
  - Argmax: DVE pairwise-max tournament s->u1(2048)->u2(1024) runs at the
    2x all-SBUF TensorTensor rate; max8+find_index8 then scan only u2.
    The winning position j in u2 maps to candidates {j,+1024,+2048,+3072};
    a GPSIMD indirect_copy gathers each point's 4 candidate s-values
    (group-shared index lists + one-hot diagonal extraction), and a small
    select picks the smallest matching k -- preserving jnp.argmin's
    first-min tie-break up to exact-f32 cross-quarter max ties (~0-1 pt).
  - Data-parallel: 8192 points/core, centers replicated; no collectives.
"""
import os
import numpy as np

import concourse.bass as bass
import concourse.bacc as bacc
import concourse.tile as tile
import concourse.mybir as mybir
from concourse.bass_utils import run_bass_kernel_spmd

N_CORES = 8
N_POINTS = 65536
K = 4096
F = 512
PTS_PER_CORE = N_POINTS // N_CORES      # 8192
NT = PTS_PER_CORE // 128                # 64 x-tiles per core
NFC = F // 128                          # 4 contraction chunks
HALF = K // 2                           # 2048 (4 PSUM banks)
QUAR = K // 4                           # 1024
F32 = mybir.dt.float32
F32R = mybir.dt.float32r
U16 = mybir.dt.uint16
U32 = mybir.dt.uint32

_NC = None
LAST_BR = None


def round_fp32r(a: np.ndarray) -> np.ndarray:
    """Round f32 to fp32r (e8m11): RNE to 11 mantissa bits; low 12 bits zero."""
    bits = np.ascontiguousarray(a, dtype=np.float32).view(np.uint32)
    rounded = (bits.astype(np.uint64) + 0x7FF + ((bits >> 12) & 1)) & 0xFFFFF000
    return rounded.astype(np.uint32).view(np.float32)


def _build():
    nc = bacc.Bacc("TRN2", target_bir_lowering=False, debug=False,
                   num_devices=N_CORES)
    xh_d = nc.declare_dram_parameter("xh", [NT, 128, NFC, 128], F32R, isOutput=False)
    ch_d = nc.declare_dram_parameter("ch", [128, NFC, K], F32R, isOutput=False)
    cnn_d = nc.declare_dram_parameter("cnn", [128, K], F32, isOutput=False)
    oh_d = nc.declare_dram_parameter("oneh", [128, 16], F32, isOutput=False)
    cv_d = nc.declare_dram_parameter("cvec", [128, 4], U16, isOutput=False)
    out_d = nc.declare_dram_parameter("oidx", [128, NT], U16, isOutput=True)

    with tile.TileContext(nc) as tc:
        with (
            tc.tile_pool(name="const", bufs=1) as cpool,
            tc.tile_pool(name="xp", bufs=3) as xpool,
            tc.tile_pool(name="sp", bufs=3) as spool,
            tc.tile_pool(name="up", bufs=2) as upool,
            tc.tile_pool(name="mp", bufs=2) as mpool,
            tc.tile_pool(name="gp", bufs=2) as gpool,
            tc.tile_pool(name="st", bufs=1) as stpool,
            tc.tile_pool(name="ps", bufs=4, space="PSUM") as pspool,
        ):
            ch = cpool.tile([128, NFC, K], F32R, tag="ch")
            cnn = cpool.tile([128, K], F32, tag="cnn")
            oneh = cpool.tile([128, 16], F32, tag="oneh")
            cvec = cpool.tile([128, 4], U16, tag="cvec")
            # spread the big ch load over four DMA queues so tile-0 matmuls
            # start after ~1/4 of it instead of all of it
            dma_engines = [nc.sync, nc.scalar, nc.gpsimd, nc.sync]
            for fc in range(NFC):
                dma_engines[fc].dma_start(ch[:, fc], ch_d[:, fc])
            nc.sync.dma_start(cnn[:], cnn_d[:])
            nc.sync.dma_start(oneh[:], oh_d[:])
            nc.sync.dma_start(cvec[:], cv_d[:])

            fi8 = stpool.tile([128, NT, 8], U16, tag="fi8")
            mst = stpool.tile([128, NT, 8], F32, tag="mst")
            g4st = stpool.tile([128, NT, 4], F32, tag="g4st")
            i4st = stpool.tile([128, NT, 4], U16, tag="i4st")
            kout = stpool.tile([128, NT], U16, tag="kout")

            # Prime has_written bits of all 8 PSUM banks (values overwritten
            # by the first prefill; only the set bits matter so start=False
            # matmuls accumulate onto the prefilled bias).
            for _ in range(4):
                prime = pspool.tile([128, QUAR], F32, tag="p")
                for b in range(2):
                    nc.tensor.matmul(
                        prime[:, b * 512:(b + 1) * 512],
                        ch[:, 0, 0:128],
                        ch[:, 0, 0:512],
                        start=True, stop=True,
                    )

            for t in range(NT):
                xh = xpool.tile([128, NFC * 128], F32R, tag="xh")
                nc.sync.dma_start(xh[:], xh_d[t])

                s = spool.tile([128, K], F32, tag="s")
                for q in range(4):
                    qs = slice(q * QUAR, (q + 1) * QUAR)
                    p = pspool.tile([128, QUAR], F32, tag="p")
                    nc.scalar.activation(
                        out=p[:], in_=cnn[:, qs],
                        func=mybir.ActivationFunctionType.Copy,
                    )
                    for fc in range(NFC):
                        for b in range(2):
                            ks = slice(q * QUAR + b * 512,
                                       q * QUAR + (b + 1) * 512)
                            nc.tensor.matmul(
                                p[:, b * 512:(b + 1) * 512],
                                xh[:, fc * 128:(fc + 1) * 128],
                                ch[:, fc, ks],
                                start=False,
                                stop=(fc == NFC - 1),
                                skip_group_check=True,
                            )
                    # egress: ACT copies q0-q2; DVE takes all of q3
                    if q < 3:
                        nc.scalar.activation(
                            out=s[:, qs], in_=p[:],
                            func=mybir.ActivationFunctionType.Copy,
                        )
                    else:
                        nc.vector.tensor_copy(s[:, 3 * QUAR:K], p[:])

                # DVE tournament at the 2x all-SBUF TT rate
                u1 = upool.tile([128, HALF], F32, tag="u1")
                nc.vector.tensor_tensor(
                    out=u1[:], in0=s[:, 0:HALF], in1=s[:, HALF:K],
                    op=mybir.AluOpType.max,
                )
                u2 = upool.tile([128, QUAR], F32, tag="u2")
                nc.vector.tensor_tensor(
                    out=u2[:], in0=u1[:, 0:QUAR], in1=u1[:, QUAR:HALF],
                    op=mybir.AluOpType.max,
                )
                nc.vector.max(mst[:, t, :], u2[:])
                nc.vector.max_index(fi8[:, t, :], mst[:, t, :], u2[:])

                # index reconstruction: candidates k = j* + 1024*c
                nc.vector.tensor_tensor(
                    out=i4st[:, t, :],
                    in0=fi8[:, t, 0:1].to_broadcast([128, 4]),
                    in1=cvec[:], op=mybir.AluOpType.add,
                )
                g64 = gpool.tile([128, 64], F32, tag="g64")
                nc.gpsimd.indirect_copy(g64[:], s[:], i4st[:, t, :], True)
                # one-hot diagonal extraction: g4[p,c] = g64[p, 16c + p%16]
                z = gpool.tile([128, 4, 16], F32, tag="z")
                nc.gpsimd.tensor_tensor(
                    out=z[:], in0=g64[:].rearrange("p (c i) -> p c i", c=4),
                    in1=oneh[:].unsqueeze(1).to_broadcast([128, 4, 16]),
                    op=mybir.AluOpType.mult,
                )
                nc.vector.tensor_reduce(g4st[:, t, :], z[:],
                                        mybir.AxisListType.X,
                                        mybir.AluOpType.add)

            # batched select over all tiles: smallest c with g4 == gmax
            nef = stpool.tile([128, NT, 4], F32, tag="nef")
            nc.vector.tensor_tensor(
                out=nef[:], in0=g4st[:],
                in1=mst[:, :, 0:1].to_broadcast([128, NT, 4]),
                op=mybir.AluOpType.is_lt,
            )
            pen = stpool.tile([128, NT, 4], U16, tag="pen")
            nc.vector.tensor_scalar(out=pen[:], in0=nef[:], scalar1=61440.0,
                                    scalar2=None, op0=mybir.AluOpType.mult)
            cand = stpool.tile([128, NT, 4], U16, tag="cand")
            nc.vector.tensor_tensor(out=cand[:], in0=i4st[:], in1=pen[:],
                                    op=mybir.AluOpType.add)
            nc.vector.tensor_reduce(kout[:], cand[:], mybir.AxisListType.X,
                                    mybir.AluOpType.min)

            nc.gpsimd.dma_start(out_d[:], kout[:])
    nc.compile()
    return nc


def _get_nc():
    global _NC
    if _NC is None:
        _NC = _build()
    return _NC


def kernel(x: np.ndarray, centers: np.ndarray) -> np.ndarray:
    global LAST_BR
    x = np.ascontiguousarray(x, dtype=np.float32)
    centers = np.ascontiguousarray(centers, dtype=np.float32)

    # +2x (not -2x): the -||c||^2 prefill is ADDED to by the matmuls.
    v_hi = round_fp32r((2.0 * x).astype(np.float32))
    c_hi = round_fp32r(centers)

    a = v_hi.reshape(N_CORES, NT, 128, NFC, 128)         # [core, t, j, fc, fp]
    xh_p = np.ascontiguousarray(a.transpose(0, 1, 4, 3, 2))

    b = c_hi.reshape(K, NFC, 128)                        # [k, fc, fp]
    ch_p = np.ascontiguousarray(b.transpose(2, 1, 0))

    c_norm = (centers.astype(np.float64) ** 2).sum(axis=1).astype(np.float32)
    cnn_p = np.ascontiguousarray(
        np.broadcast_to(-c_norm[None, :], (128, K)).astype(np.float32))

    oh_p = np.zeros((128, 16), dtype=np.float32)
    oh_p[np.arange(128), np.arange(128) % 16] = 1.0
    cv_p = np.ascontiguousarray(
        np.broadcast_to((np.arange(4, dtype=np.uint16) * QUAR)[None, :],
                        (128, 4)))

    in_maps = [
        {"xh": xh_p[i], "ch": ch_p, "cnn": cnn_p, "oneh": oh_p, "cvec": cv_p}
        for i in range(N_CORES)
    ]

    nc = _get_nc()
    global _LAST_IN_MAPS
    _LAST_IN_MAPS = in_maps
    br = run_bass_kernel_spmd(nc, in_maps, list(range(N_CORES)))
    LAST_BR = br

    parts = []
    for i in range(N_CORES):
        oidx = br.results[i]["oidx"]                      # (128, NT) u32
        parts.append(oidx.T.reshape(-1))                  # point-major
    return np.concatenate(parts).astype(np.int32)


_LAST_IN_MAPS = None


# revision 26
# speedup vs baseline: 1.0044x; 1.0044x over previous
"""KMeans assignment kernel for TRN2 (8 NeuronCores, data-parallel over points).

Computes argmin_k ||x_n - c_k||^2 for x (65536, 512) f32, centers (4096, 512) f32.

v5 strategy (single-pass fp32r + PSUM-prefill bias + tournament argmax):
  - argmin_k dist = argmax_k s,  s = 2x.c_k - ||c_k||^2  (x-norm constant/row).
  - ONE fp32r matmul pass (e8m11 operands): ~13/65536 argmin flips
    (rel err ~9e-3, gate 2e-2) at 3x less PE work than exact 3-pass.
  - -||c||^2 is PREFILLED into PSUM by ScalarE; matmuls accumulate onto it
    (start=False; has_written bits primed once), so s materializes in PSUM.
  - Egress PSUM->SBUF split between ScalarE and VectorE for balance.
  - Argmax: DVE pairwise-max tournament s->u1(2048)->u2(1024) runs at the
    2x all-SBUF TensorTensor rate; max8+find_index8 then scan only u2.
    The winning position j in u2 maps to candidates {j,+1024,+2048,+3072};
    a GPSIMD indirect_copy gathers each point's 4 candidate s-values
    (group-shared index lists + one-hot diagonal extraction), and a small
    select picks the smallest matching k -- preserving jnp.argmin's
    first-min tie-break up to exact-f32 cross-quarter max ties (~0-1 pt).
  - Data-parallel: 8192 points/core, centers replicated; no collectives.
"""
import os
import numpy as np

import concourse.bass as bass
import concourse.bacc as bacc
import concourse.tile as tile
import concourse.mybir as mybir
from concourse.bass_utils import run_bass_kernel_spmd

N_CORES = 8
N_POINTS = 65536
K = 4096
F = 512
PTS_PER_CORE = N_POINTS // N_CORES      # 8192
NT = PTS_PER_CORE // 128                # 64 x-tiles per core
NFC = F // 128                          # 4 contraction chunks
HALF = K // 2                           # 2048 (4 PSUM banks)
QUAR = K // 4                           # 1024
F32 = mybir.dt.float32
F32R = mybir.dt.float32r
U16 = mybir.dt.uint16
U32 = mybir.dt.uint32

_NC = None
LAST_BR = None


def round_fp32r(a: np.ndarray) -> np.ndarray:
    """Round f32 to fp32r (e8m11): RNE to 11 mantissa bits; low 12 bits zero."""
    bits = np.ascontiguousarray(a, dtype=np.float32).view(np.uint32)
    rounded = (bits.astype(np.uint64) + 0x7FF + ((bits >> 12) & 1)) & 0xFFFFF000
    return rounded.astype(np.uint32).view(np.float32)


def _build():
    nc = bacc.Bacc("TRN2", target_bir_lowering=False, debug=False,
                   num_devices=N_CORES)
    xh_d = nc.declare_dram_parameter("xh", [NT, 128, NFC, 128], F32R, isOutput=False)
    ch_d = nc.declare_dram_parameter("ch", [128, NFC, K], F32R, isOutput=False)
    cnn_d = nc.declare_dram_parameter("cnn", [128, K], F32, isOutput=False)
    oh_d = nc.declare_dram_parameter("oneh", [128, 16], F32, isOutput=False)
    cv_d = nc.declare_dram_parameter("cvec", [128, 4], U16, isOutput=False)
    out_d = nc.declare_dram_parameter("oidx", [128, NT], U16, isOutput=True)

    with tile.TileContext(nc) as tc:
        with (
            tc.tile_pool(name="const", bufs=1) as cpool,
            tc.tile_pool(name="xp", bufs=3) as xpool,
            tc.tile_pool(name="sp", bufs=3) as spool,
            tc.tile_pool(name="up", bufs=2) as upool,
            tc.tile_pool(name="mp", bufs=2) as mpool,
            tc.tile_pool(name="gp", bufs=2) as gpool,
            tc.tile_pool(name="st", bufs=1) as stpool,
            tc.tile_pool(name="ps", bufs=4, space="PSUM") as pspool,
        ):
            ch = cpool.tile([128, NFC, K], F32R, tag="ch")
            cnn = cpool.tile([128, K], F32, tag="cnn")
            oneh = cpool.tile([128, 16], F32, tag="oneh")
            cvec = cpool.tile([128, 4], U16, tag="cvec")
            # cnn + small constants first on their own queue (the first
            # prefill only needs cnn); spread ch over the other queues so
            # priming + tile-0 matmuls start after ~1/4 of it
            nc.scalar.dma_start(cnn[:], cnn_d[:])
            nc.scalar.dma_start(oneh[:], oh_d[:])
            nc.scalar.dma_start(cvec[:], cv_d[:])
            dma_engines = [nc.sync, nc.gpsimd, nc.scalar, nc.gpsimd]
            for fc in range(NFC):
                dma_engines[fc].dma_start(ch[:, fc], ch_d[:, fc])

            fi8 = stpool.tile([128, NT, 8], U16, tag="fi8")
            mst = stpool.tile([128, NT, 8], F32, tag="mst")
            g4st = stpool.tile([128, NT, 4], F32, tag="g4st")
            i4st = stpool.tile([128, NT, 4], U16, tag="i4st")
            kout = stpool.tile([128, NT], U16, tag="kout")

            # Prime has_written bits of all 8 PSUM banks (values overwritten
            # by the first prefill; only the set bits matter so start=False
            # matmuls accumulate onto the prefilled bias).
            for _ in range(4):
                prime = pspool.tile([128, QUAR], F32, tag="p")
                for b in range(2):
                    nc.tensor.matmul(
                        prime[:, b * 512:(b + 1) * 512],
                        ch[:, 0, 0:128],
                        ch[:, 0, 0:512],
                        start=True, stop=True,
                    )

            for t in range(NT):
                xh = xpool.tile([128, NFC * 128], F32R, tag="xh")
                nc.sync.dma_start(xh[:], xh_d[t])

                s = spool.tile([128, K], F32, tag="s")
                for q in range(4):
                    qs = slice(q * QUAR, (q + 1) * QUAR)
                    p = pspool.tile([128, QUAR], F32, tag="p")
                    nc.scalar.activation(
                        out=p[:], in_=cnn[:, qs],
                        func=mybir.ActivationFunctionType.Copy,
                    )
                    for fc in range(NFC):
                        for b in range(2):
                            ks = slice(q * QUAR + b * 512,
                                       q * QUAR + (b + 1) * 512)
                            nc.tensor.matmul(
                                p[:, b * 512:(b + 1) * 512],
                                xh[:, fc * 128:(fc + 1) * 128],
                                ch[:, fc, ks],
                                start=False,
                                stop=(fc == NFC - 1),
                                skip_group_check=True,
                            )
                    # egress: ACT copies q0-q2; DVE takes all of q3
                    if q < 3:
                        nc.scalar.activation(
                            out=s[:, qs], in_=p[:],
                            func=mybir.ActivationFunctionType.Copy,
                        )
                    else:
                        nc.vector.tensor_copy(s[:, 3 * QUAR:K], p[:])

                # DVE tournament at the 2x all-SBUF TT rate
                u1 = upool.tile([128, HALF], F32, tag="u1")
                nc.vector.tensor_tensor(
                    out=u1[:], in0=s[:, 0:HALF], in1=s[:, HALF:K],
                    op=mybir.AluOpType.max,
                )
                u2 = upool.tile([128, QUAR], F32, tag="u2")
                nc.vector.tensor_tensor(
                    out=u2[:], in0=u1[:, 0:QUAR], in1=u1[:, QUAR:HALF],
                    op=mybir.AluOpType.max,
                )
                nc.vector.max(mst[:, t, :], u2[:])
                nc.vector.max_index(fi8[:, t, :], mst[:, t, :], u2[:])

                # index reconstruction: candidates k = j* + 1024*c
                nc.vector.tensor_tensor(
                    out=i4st[:, t, :],
                    in0=fi8[:, t, 0:1].to_broadcast([128, 4]),
                    in1=cvec[:], op=mybir.AluOpType.add,
                )
                g64 = gpool.tile([128, 64], F32, tag="g64")
                nc.gpsimd.indirect_copy(g64[:], s[:], i4st[:, t, :], True)
                # one-hot diagonal extraction: g4[p,c] = g64[p, 16c + p%16]
                z = gpool.tile([128, 4, 16], F32, tag="z")
                nc.gpsimd.tensor_tensor(
                    out=z[:], in0=g64[:].rearrange("p (c i) -> p c i", c=4),
                    in1=oneh[:].unsqueeze(1).to_broadcast([128, 4, 16]),
                    op=mybir.AluOpType.mult,
                )
                nc.vector.tensor_reduce(g4st[:, t, :], z[:],
                                        mybir.AxisListType.X,
                                        mybir.AluOpType.add)

            # batched select over all tiles: smallest c with g4 == gmax
            nef = stpool.tile([128, NT, 4], F32, tag="nef")
            nc.vector.tensor_tensor(
                out=nef[:], in0=g4st[:],
                in1=mst[:, :, 0:1].to_broadcast([128, NT, 4]),
                op=mybir.AluOpType.is_lt,
            )
            pen = stpool.tile([128, NT, 4], U16, tag="pen")
            nc.vector.tensor_scalar(out=pen[:], in0=nef[:], scalar1=61440.0,
                                    scalar2=None, op0=mybir.AluOpType.mult)
            cand = stpool.tile([128, NT, 4], U16, tag="cand")
            nc.vector.tensor_tensor(out=cand[:], in0=i4st[:], in1=pen[:],
                                    op=mybir.AluOpType.add)
            nc.vector.tensor_reduce(kout[:], cand[:], mybir.AxisListType.X,
                                    mybir.AluOpType.min)

            nc.gpsimd.dma_start(out_d[:], kout[:])
    nc.compile()
    return nc


def _get_nc():
    global _NC
    if _NC is None:
        _NC = _build()
    return _NC


def kernel(x: np.ndarray, centers: np.ndarray) -> np.ndarray:
    global LAST_BR
    x = np.ascontiguousarray(x, dtype=np.float32)
    centers = np.ascontiguousarray(centers, dtype=np.float32)

    # +2x (not -2x): the -||c||^2 prefill is ADDED to by the matmuls.
    v_hi = round_fp32r((2.0 * x).astype(np.float32))
    c_hi = round_fp32r(centers)

    a = v_hi.reshape(N_CORES, NT, 128, NFC, 128)         # [core, t, j, fc, fp]
    xh_p = np.ascontiguousarray(a.transpose(0, 1, 4, 3, 2))

    b = c_hi.reshape(K, NFC, 128)                        # [k, fc, fp]
    ch_p = np.ascontiguousarray(b.transpose(2, 1, 0))

    c_norm = (centers.astype(np.float64) ** 2).sum(axis=1).astype(np.float32)
    cnn_p = np.ascontiguousarray(
        np.broadcast_to(-c_norm[None, :], (128, K)).astype(np.float32))

    oh_p = np.zeros((128, 16), dtype=np.float32)
    oh_p[np.arange(128), np.arange(128) % 16] = 1.0
    cv_p = np.ascontiguousarray(
        np.broadcast_to((np.arange(4, dtype=np.uint16) * QUAR)[None, :],
                        (128, 4)))

    in_maps = [
        {"xh": xh_p[i], "ch": ch_p, "cnn": cnn_p, "oneh": oh_p, "cvec": cv_p}
        for i in range(N_CORES)
    ]

    nc = _get_nc()
    global _LAST_IN_MAPS
    _LAST_IN_MAPS = in_maps
    br = run_bass_kernel_spmd(nc, in_maps, list(range(N_CORES)))
    LAST_BR = br

    parts = []
    for i in range(N_CORES):
        oidx = br.results[i]["oidx"]                      # (128, NT) u32
        parts.append(oidx.T.reshape(-1))                  # point-major
    return np.concatenate(parts).astype(np.int32)


_LAST_IN_MAPS = None
